# revision 48
# baseline (speedup 1.0000x reference)
"""DirGATConv on 8 Trainium2 NeuronCores (Bass/Tile), v3.

Problem: nn_DirGATConv  (N=50000 nodes, E=800000 edges, DIN=128, DOUT=64)
    out = 0.5 * GATConv(x, src->dst, W1) + 0.5 * GATConv(x, dst->src, W2)

The kernel is gather-descriptor bound (SWDGE runs on 4 Pool queues with
~2 DMA engines each; per-descriptor cost barely depends on 256B vs 512B
element size), so the design minimizes gather descriptors:
  * Self-edges (incl. the PyG-added self-loop) never hit the gather path:
    their messages are fully host-computed (slf input) and seeded into the
    psum with one eye-matmul per (chunk, conv).
  * Each core gets its own table-row permutation (per-core xT column
    order).  A per-core greedy matching pairs sources that share a
    destination in either conv onto adjacent rows (2k, 2k+1), so one 512B
    pair-descriptor serves TWO edges (both halves active); ~12% of cells
    merge this way.
  * a_d[dst] is computed on host (f64) and folded into the par table, so
    the device never materializes attention-dst terms; par is a_d for an
    active half and -200 (kills exp) for inactive/padding halves.
  * Phase A (replicated): one fused matmul per chunk computes
    [xw1 | . | a_s1 | xw2 | . | a_s2]; ACT/DVE copies convert PSUM->f16
    staging; tables stored row-permuted (row = (col%128)*G + col//128) so
    staged writes are >=512B/descriptor.  Row = [xw(64) | 1 | a_s | junk].
  * Phase B: per (chunk, conv) an int16 index space addresses row PAIRS
    (idx = row>>1, elem 512B).  Each dst's first q cells sit in "diagonal"
    slots (partition = dst local id, lhsT = I); leftovers go to one-hot
    "tail" blocks.  u = a_s(gathered) + par; exp(leaky_relu(u)) runs on
    ACT over the per-cell scalars only; messages [w*xw | w] accumulate on
    the PE into PSUM [128, 65] per conv; column 64 is the softmax
    denominator.
"""

import math

import numpy as np

import concourse.bass as bass
import concourse.mybir as mybir
import concourse.tile as tile
from concourse import bacc, bass_utils
from concourse._compat import with_exitstack

# ---------------------------------------------------------------- constants
N = 50000
E = 800000
DIN = 128
DOUT = 64
ALPHA = 0.5
NEG_SLOPE = 0.2
NCORES = 8
P = 128

G = NCORES * math.ceil(math.ceil(N / P) / NCORES)  # 392 padded chunks
CPC = G // NCORES                                  # 49 chunk slots per core
NT = G * P                                         # 50176 padded node count
NPC = CPC * P                                      # 6272 nodes per core

import os

WCOLS = 132        # wfull columns: 2 convs x [W(64) | 1 | a_s]
RNDA = 3           # chunks per phase-A psum round (3*132*4B < 2KB bank)
STG = 12           # chunks per staging flush
SCN = int(os.environ.get("K_SCN", 2))    # chunks per gather super-tile
QCAP = int(os.environ.get("K_QCAP", 8))  # blocks per gather call
SCRATCH = int(os.environ.get("K_SCRATCH", 16384))
                   # SWDGE descriptor carveout bytes (1024 descs);
                   # larger values fail to execute under this runtime
NQ = int(os.environ.get("K_NQ", 4))      # SWDGE queues, round-robin
SP = os.environ.get("K_SP", "1") == "1"  # dma_gather single_packet
PGB = int(os.environ.get("K_PGB", 2))    # gather tile pool bufs

f32 = mybir.dt.float32
f16 = mybir.dt.float16
i16 = mybir.dt.int16

VARIANT = "full"    # "full" | "gathers" | "phasea"  (perf-bisect variants)

_CACHE = {}


# ------------------------------------------------------------ host preprocess
def _match_cores(dsts, srcs):
    """Per-core greedy pairing of sources that share a destination (over
    both convs).  Matched pairs get adjacent table rows (2k, 2k+1) so one
    512B pair-descriptor serves two edges.  Returns (rowOf [NCORES, NT],
    nodeAt [NCORES, NT]): table row of each node / node at each xT column,
    per core."""
    key_all = np.concatenate([dsts, srcs + NT])      # disjoint group spaces
    gidx_all = np.concatenate([srcs, dsts])
    core_all = (np.concatenate([dsts, srcs]) // P) // CPC
    rowOf = np.empty((NCORES, NT), np.int64)
    nodeAt = np.empty((NCORES, NT), np.int64)
    jcols = np.arange(G * P)
    rows_of_col = (jcols % P) * G + jcols // P
    for c in range(NCORES):
        m = core_all == c
        g, s = key_all[m], gidx_all[m]
        order = np.lexsort((s, g))
        gs, ss = g[order], s[order]
        starts = np.flatnonzero(np.r_[True, gs[1:] != gs[:-1]])
        ends = np.r_[starts[1:], len(gs)]
        pa, pb = [], []
        for st, en in zip(starts, ends):
            if en - st < 2:
                continue
            mem = np.unique(ss[st:en])
            if len(mem) < 2:
                continue
            ii, jj = np.triu_indices(len(mem), 1)
            pa.append(mem[ii])
            pb.append(mem[jj])
        A = np.empty(0, np.int64)
        B = np.empty(0, np.int64)
        if pa:
            pa = np.concatenate(pa)
            pb = np.concatenate(pb)
            pk = np.minimum(pa, pb) * np.int64(NT) + np.maximum(pa, pb)
            up, cnt = np.unique(pk, return_counts=True)
            o = np.argsort(-cnt, kind="stable")
            la, lb = up[o] // NT, up[o] % NT
            used = np.zeros(NT, bool)
            keep = np.zeros(len(la), bool)
            for i in range(len(la)):
                a, b = la[i], lb[i]
                if not (used[a] or used[b]):
                    used[a] = used[b] = True
                    keep[i] = True
            A, B = la[keep], lb[keep]
        npair = len(A)
        i = np.arange(npair)
        p, gp = i % P, i // P
        na = np.empty(G * P, np.int64)
        taken = np.zeros(G * P, bool)
        ca = (2 * gp) * P + p
        cb = (2 * gp + 1) * P + p
        na[ca] = A
        na[cb] = B
        taken[ca] = True
        taken[cb] = True
        isn = np.zeros(NT, bool)
        isn[A] = True
        isn[B] = True
        na[np.flatnonzero(~taken)] = np.flatnonzero(~isn)
        nodeAt[c] = na
        rowOf[c, na] = rows_of_col
    return rowOf, nodeAt


def _prep_conv(key, gidx, adv, rowOf):
    """Edge layout for one conv.  key = group node (output row), gidx =
    gathered node, adv = host-computed a_d per node (folded into par),
    rowOf = per-core table row of each node.  Edges sharing (dst, row-pair)
    merge into one cell (both 256B halves active).
    Returns (q, kbt, per-core dict arrays)."""
    key = np.asarray(key, np.int64)
    gidx = np.asarray(gidx, np.int64)
    ecore = (key // P) // CPC
    row = rowOf[ecore, gidx]
    kpr = row >> 1
    parity = (row & 1).astype(np.int64)

    # rank within (key, kpr, parity) splits duplicate edges into cells
    okey = (key * 32768 + kpr) * 2 + parity
    order = np.argsort(okey, kind="stable")
    so = okey[order]
    newrun = np.r_[True, so[1:] != so[:-1]]
    runid = np.cumsum(newrun) - 1
    rstarts = np.flatnonzero(newrun)
    rank = np.empty(len(okey), np.int64)
    rank[order] = np.arange(len(so)) - rstarts[runid]
    RMAX = int(rank.max()) + 1

    ckey = (key * 32768 + kpr) * RMAX + rank
    ucells, inv = np.unique(ckey, return_inverse=True)
    cdst = ucells // (32768 * RMAX)
    cpair = (ucells // RMAX) % 32768
    chunk = cdst // P
    dloc = cdst % P
    core = chunk // CPC
    slot = chunk % CPC

    D = np.bincount(chunk * P + dloc, minlength=G * P).reshape(
        NCORES, CPC, P)

    TAILW = 0.3
    q = np.zeros(CPC, np.int64)
    kbt = np.zeros(CPC, np.int64)
    for s in range(CPC):
        Ds = D[:, s, :]
        best = None
        for qq in range(0, int(Ds.max()) + 1):
            tails = np.maximum(Ds - qq, 0).sum(axis=1)
            kb = int(np.ceil(tails / P).max())
            cost = qq + (1.0 + TAILW) * kb
            if best is None or cost < best[0] or (
                    cost == best[0] and qq > best[1]):
                best = (cost, qq, kb)
        q[s], kbt[s] = best[1], best[2]

    nb = q + kbt                       # blocks per (slot, conv)
    boff = np.zeros(CPC + 1, np.int64)
    boff[1:] = np.cumsum(nb)
    tboff = np.zeros(CPC + 1, np.int64)
    tboff[1:] = np.cumsum(kbt)
    NB = int(boff[-1])
    KT = int(tboff[-1])

    # rank of each cell within its (chunk, dloc) group
    ck = chunk * P + dloc
    order = np.argsort(ck, kind="stable")
    counts = np.bincount(ck, minlength=G * P)
    gstart = np.zeros(G * P, np.int64)
    gstart[1:] = np.cumsum(counts)[:-1]
    crank = np.empty(len(ck), np.int64)
    crank[order] = np.arange(len(ck)) - gstart[ck[order]]

    qs_e = q[slot]
    isdiag = crank < qs_e

    tkey = core * CPC + slot
    torder = np.argsort(np.where(isdiag, -1, tkey), kind="stable")
    tsorted = torder[int(isdiag.sum()):]          # tail cells, grouped
    tcounts = np.bincount(tkey[tsorted], minlength=NCORES * CPC)
    tstart = np.zeros(NCORES * CPC, np.int64)
    tstart[1:] = np.cumsum(tcounts)[:-1]
    trank = np.zeros(len(ck), np.int64)
    trank[tsorted] = np.arange(len(tsorted)) - tstart[tkey[tsorted]]

    blk = np.where(isdiag, crank, qs_e + trank // P)
    lane = np.where(isdiag, dloc, trank % P)
    cgpos = (boff[slot] + blk) * P + lane

    TOT = NB * P
    ix = np.zeros((NCORES, TOT), np.int16)
    par = np.full((NCORES, TOT, 2), -200.0, np.float16)
    dlt = np.zeros((NCORES, P, max(KT, 1)), np.float16)

    ix[core, cgpos] = cpair.astype(np.int16)
    par[ecore, cgpos[inv], parity] = adv[key]
    tm = ~isdiag
    dlt[core[tm], lane[tm], tboff[slot[tm]] + trank[tm] // P] = \
        dloc[tm].astype(np.float16)

    # wrap indices for dma_gather: [128, TOT//16]
    ixw = ix.reshape(NCORES, TOT // 16, 16).transpose(0, 2, 1)
    ixw = np.ascontiguousarray(np.tile(ixw, (1, 8, 1)))
    # par device layout [128, NB, 2]
    parw = np.ascontiguousarray(
        par.reshape(NCORES, NB, P, 2).transpose(0, 2, 1, 3)
        .reshape(NCORES, P, NB * 2))
    return (tuple(int(v) for v in q), tuple(int(v) for v in kbt),
            ixw, parw, dlt)


def _preprocess(x, edge_index, W1, att_src1, att_dst1, b1,
                W2, att_src2, att_dst2, b2):
    src = np.asarray(edge_index[0], np.int64)
    dst = np.asarray(edge_index[1], np.int64)
    # self-edges (incl. the PyG-added self-loop) are handled host-side via
    # slf; only cross edges go through the gather path.
    cross = src != dst
    nself = np.bincount(src[~cross], minlength=N)  # real (n,n) multiplicity
    src, dst = src[cross], dst[cross]

    x64 = np.asarray(x, np.float64)
    W1_, W2_ = np.asarray(W1, np.float64), np.asarray(W2, np.float64)
    xw1 = x64 @ W1_
    xw2 = x64 @ W2_
    as1 = xw1 @ np.asarray(att_src1, np.float64)
    as2 = xw2 @ np.asarray(att_src2, np.float64)
    ad1 = xw1 @ np.asarray(att_dst1, np.float64)
    ad2 = xw2 @ np.asarray(att_dst2, np.float64)

    rowOf, nodeAt = _match_cores(dst, src)
    q1, kbt1, ixw1, parw1, dlt1 = _prep_conv(dst, src, ad1, rowOf)
    q2, kbt2, ixw2, parw2, dlt2 = _prep_conv(src, dst, ad2, rowOf)

    # per-node self message  (1+m) * exp(lrelu(a_s+a_d)) * [xw | 1]
    def lrelu(v):
        return np.where(v > 0, v, NEG_SLOPE * v)

    slf = np.zeros((NCORES, P, CPC, 2, 66), np.float16)
    for cv, (xw, a_s, a_d) in enumerate(((xw1, as1, ad1), (xw2, as2, ad2))):
        ws = (1.0 + nself) * np.exp(lrelu(a_s + a_d))        # [N]
        m = np.zeros((NT, 66))
        m[:N, :64] = ws[:, None] * xw
        m[:N, 64] = ws
        mg = m.reshape(G, P, 66).transpose(1, 0, 2)          # [P, G, 66]
        slf[:, :, :, cv, :] = mg.reshape(
            P, NCORES, CPC, 66).transpose(1, 0, 2, 3).astype(np.float16)

    xT0 = np.zeros((DIN, NT), np.float16)
    xT0[:, :N] = np.asarray(x, np.float32).T.astype(np.float16)

    wfull = np.zeros((DIN, WCOLS), np.float64)
    for cv, (W, a_s) in enumerate(((W1, att_src1), (W2, att_src2))):
        o = cv * 66
        wfull[:, o:o + 64] = W
        wfull[:, o + 65] = W @ a_s
    wfull = wfull.astype(np.float16)

    iota = np.broadcast_to(np.arange(P, dtype=np.float16), (P, P)).copy()
    eye = np.eye(P, dtype=np.float16)
    bcomb = np.broadcast_to(
        ((1.0 - ALPHA) * np.asarray(b1, np.float64)
         + ALPHA * np.asarray(b2, np.float64)).astype(np.float32),
        (P, DOUT)).copy()

    common = dict(wfull=wfull, iota=iota, eye=eye, bcomb=bcomb)
    per_core = []
    for k in range(NCORES):
        per_core.append(dict(
            xT=np.ascontiguousarray(xT0[:, nodeAt[k]]),
            ix1=ixw1[k], ix2=ixw2[k],
            par1=parw1[k], par2=parw2[k],
            dlt1=dlt1[k], dlt2=dlt2[k],
            slf=np.ascontiguousarray(slf[k].reshape(P, CPC * 2 * 66))))
    meta = (q1, kbt1, q2, kbt2)
    return common, per_core, meta


# ------------------------------------------------------------- device program
@with_exitstack
def _emit(ctx, tc, outs, ins, meta, reps=1):
    nc = tc.nc
    t1_d = nc.dram_tensor("T1_tab", [NT, P], f16, kind="Internal").ap()
    t2_d = nc.dram_tensor("T2_tab", [NT, P], f16, kind="Internal").ap()
    for _ in range(reps):
        _emit_iter(tc, outs, ins, meta, t1_d, t2_d)


def _emit_iter(tc, outs, ins, meta, t1_d, t2_d):
    nc = tc.nc
    out_d = outs["out"]
    q1, kbt1, q2, kbt2 = meta
    qs_ = (q1, q2)
    kbt_ = (kbt1, kbt2)
    nb_ = tuple(tuple(a + b for a, b in zip(qs_[c], kbt_[c]))
                for c in range(2))
    boff_ = []
    tboff_ = []
    for c in range(2):
        bo = [0]
        to = [0]
        for s in range(CPC):
            bo.append(bo[-1] + nb_[c][s])
            to.append(to[-1] + kbt_[c][s])
        boff_.append(bo)
        tboff_.append(to)
    NBMAX = max(max(nb_[0]), max(nb_[1]))
    KTMAX = max(max(kbt_[0]), max(kbt_[1]), 1)

    t_views = [t.rearrange("(p g) c -> p g c", p=P) for t in (t1_d, t2_d)]
    t_pair = [t.rearrange("(r t) c -> r (t c)", t=2) for t in (t1_d, t2_d)]

    # ---------------- phase A: tables ----------------
    with tc.tile_pool(name="pa0", bufs=1) as pa0, \
         tc.tile_pool(name="pa", bufs=2) as pa, \
         tc.tile_pool(name="pastg", bufs=2) as pastg, \
         tc.tile_pool(name="pap", bufs=2, space="PSUM") as pap:
        wf = pa0.tile([P, WCOLS], f16)
        nc.sync.dma_start(out=wf[:], in_=ins["wfull"][:])

        for piece in range(NCORES):
            xt = pa.tile([P, NPC], f16, tag="xt")
            # split the load across HWDGE and SWDGE paths (balances phase-A
            # DMA: each path ends up with ~19MB incl. the staging writes)
            h = NPC // 2
            nc.sync.dma_start(
                out=xt[:, :h],
                in_=ins["xT"][:, piece * NPC:piece * NPC + h])
            nc.gpsimd.dma_start(
                out=xt[:, h:],
                in_=ins["xT"][:, piece * NPC + h:(piece + 1) * NPC])
            stg = [pastg.tile([P, CPC, P], f16, tag=f"stg{t}",
                              name=f"stg{t}") for t in range(2)]
            gbase = piece * CPC
            for j0 in range(0, CPC, RNDA):
                r = min(RNDA, CPC - j0)
                ps = pap.tile([P, RNDA * WCOLS], f32, tag="pap")
                for k in range(r):
                    nc.tensor.matmul(
                        out=ps[:, k * WCOLS:(k + 1) * WCOLS],
                        lhsT=xt[:, (j0 + k) * P:(j0 + k + 1) * P],
                        rhs=wf[:], start=True, stop=True)
                psv = ps[:].rearrange("p (k c) -> p k c", k=RNDA)
                # T1 copy on ACT, T2 on DVE - balance the engines
                nc.scalar.copy(
                    out=stg[0][:, j0:j0 + r, 0:66], in_=psv[:, :r, 0:66])
                nc.vector.tensor_copy(
                    out=stg[1][:, j0:j0 + r, 0:66], in_=psv[:, :r, 66:132])
                for t in range(2):
                    nc.vector.memset(stg[t][:, j0:j0 + r, 64:65], 1.0)
            # T1 via HWDGE (SP/Act), T2 via SWDGE (Pool) - the SWDGE DMA
            # engines are idle during phase A, so this doubles write paths
            nc.sync.dma_start(
                out=t_views[0][:, gbase:gbase + CPC, :],
                in_=stg[0][:, :, :])
            nc.gpsimd.dma_start(
                out=t_views[1][:, gbase:gbase + CPC, :],
                in_=stg[1][:, :, :])

    # ---------------- phase B: edge aggregation ----------------
    with tc.tile_pool(name="pb0", bufs=1) as pb0, \
         tc.tile_pool(name="pg", bufs=PGB) as pg, \
         tc.tile_pool(name="pb", bufs=3) as pb, \
         tc.tile_pool(name="pbp", bufs=4, space="PSUM") as pbp:
        iota = pb0.tile([P, P], f16)
        nc.sync.dma_start(out=iota[:], in_=ins["iota"][:])
        eye = pb0.tile([P, P], f16)
        nc.sync.dma_start(out=eye[:], in_=ins["eye"][:])
        bcomb = pb0.tile([P, DOUT], f32)
        nc.sync.dma_start(out=bcomb[:], in_=ins["bcomb"][:])
        slft = pb0.tile([P, CPC, 2, 66], f16)
        nc.sync.dma_start(out=slft[:].rearrange("p s c v -> p (s c v)"),
                          in_=ins["slf"][:])
        dlts = []
        pars = []
        ixalls = []
        for cv in range(2):
            kt = max(tboff_[cv][-1], 1)
            t = pb0.tile([P, kt], f16, tag=f"dlt{cv}", name=f"dlt{cv}")
            nc.sync.dma_start(out=t[:], in_=ins[f"dlt{cv + 1}"][:])
            dlts.append(t)
            nbt = boff_[cv][-1]
            t = pb0.tile([P, nbt, 2], f16, tag=f"par{cv}", name=f"par{cv}")
            nc.sync.dma_start(
                out=t[:].rearrange("p b t -> p (b t)"),
                in_=ins[f"par{cv + 1}"][:])
            pars.append(t)
            # all gather indices resident up front: kills the per-super-tile
            # ixt-load dependency ahead of each gather batch
            t = pb0.tile([P, nbt * 8], i16, tag=f"ixall{cv}",
                         name=f"ixall{cv}")
            nc.sync.dma_start(out=t[:], in_=ins[f"ix{cv + 1}"][:])
            ixalls.append(t)

        if VARIANT == "phasea":
            for s in range(CPC):
                nc.sync.dma_start(out=out_d[s * P:(s + 1) * P, :],
                                  in_=bcomb[:])
            return

        rrq = [0]
        NBS = [max(sum(nb_[cv][s0:s0 + SCN]) for s0 in range(0, CPC, SCN))
               for cv in range(2)]
        t_quad = [t.rearrange("(r t) c -> r (t c)", t=4) for t in (t1_d, t2_d)]

        for s0 in range(0, CPC, SCN):
            scr = min(SCN, CPC - s0)
            gts = []
            for cv in range(2):
                nbsup = sum(nb_[cv][s0:s0 + scr])
                bo0 = boff_[cv][s0]
                gt = None
                if VARIANT not in ("g256", "g1024"):
                    gt = pg.tile([P, NBS[cv], 2, P], f16, tag=f"gt{cv}",
                                 name=f"gt{cv}")
                gv = (gt[:].rearrange("p k t c -> p k (t c)")
                      if gt is not None else None)
                if VARIANT == "g256":
                    # timing probe: same desc count, 256B elems
                    g2 = pg.tile([P, NBS[cv], P], f16, tag=f"g2{cv}",
                                 name=f"g2{cv}")
                    for b0 in range(0, nbsup, QCAP):
                        bw = min(QCAP, nbsup - b0)
                        nc.gpsimd.dma_gather(
                            out_ap=g2[:, b0:b0 + bw, :],
                            in_ap=(t1_d, t2_d)[cv],
                            idxs_ap=ixalls[cv][:, (bo0 + b0) * 8:
                                               (bo0 + b0 + bw) * 8],
                            num_idxs=bw * P, num_idxs_reg=bw * P,
                            elem_size=P, queue_num=rrq[0])
                        rrq[0] = (rrq[0] + 1) % NQ
                elif VARIANT == "g1024":
                    # timing probe: half desc count, 1024B elems.
                    # Mask idx to < NT//4 so quad-view reads stay in bounds.
                    ixm = pg.tile([P, NBS[cv] * 8], i16, tag=f"ixm{cv}",
                                  name=f"ixm{cv}")
                    nc.vector.tensor_scalar(
                        out=ixm[:, :nbsup * 8],
                        in0=ixalls[cv][:, bo0 * 8:(bo0 + nbsup) * 8],
                        scalar1=0x1FFF, scalar2=None,
                        op0=mybir.AluOpType.bitwise_and)
                    gq = pg.tile([P, NBS[cv] // 2 + 1, 4 * P], f16,
                                 tag=f"gq{cv}", name=f"gq{cv}")
                    for b0 in range(0, nbsup // 2, QCAP):
                        bw = min(QCAP, nbsup // 2 - b0)
                        nc.gpsimd.dma_gather(
                            out_ap=gq[:, b0:b0 + bw, :], in_ap=t_quad[cv],
                            idxs_ap=ixm[:, b0 * 8:(b0 + bw) * 8],
                            num_idxs=bw * P, num_idxs_reg=bw * P,
                            elem_size=4 * P, queue_num=rrq[0])
                        rrq[0] = (rrq[0] + 1) % NQ
                else:
                    for b0 in range(0, nbsup, QCAP):
                        bw = min(QCAP, nbsup - b0)
                        nc.gpsimd.dma_gather(
                            out_ap=gv[:, b0:b0 + bw, :], in_ap=t_pair[cv],
                            idxs_ap=ixalls[cv][:, (bo0 + b0) * 8:
                                               (bo0 + b0 + bw) * 8],
                            num_idxs=bw * P, num_idxs_reg=bw * P,
                            elem_size=2 * P, queue_num=rrq[0],
                            single_packet=SP)
                        rrq[0] = (rrq[0] + 1) % NQ
                gts.append(gt if VARIANT != "g256" else None)
            if VARIANT != "full":
                for s in range(s0, s0 + scr):
                    nc.sync.dma_start(out=out_d[s * P:(s + 1) * P, :],
                                      in_=bcomb[:])
                continue
            for s in range(s0, s0 + scr):
                psums = []
                for cv in range(2):
                    qs = qs_[cv][s]
                    ks = kbt_[cv][s]
                    nb = nb_[cv][s]
                    bo = boff_[cv][s]
                    to = tboff_[cv][s]
                    go = boff_[cv][s] - boff_[cv][s0]
                    gt = gts[cv]
                    # par carries host-computed a_d[dst] (or -200 disable)
                    u = pb.tile([P, NBMAX, 2], f16, tag="u")
                    nc.vector.tensor_tensor(
                        out=u[:, :nb, :], in0=gt[:, go:go + nb, :, 65],
                        in1=pars[cv][:, bo:bo + nb, :],
                        op=mybir.AluOpType.add)
                    oh = None
                    if ks:
                        oh = pb.tile([P, KTMAX, P], f16, tag="oh")
                        dlv = dlts[cv][:, to:to + ks]
                        nc.vector.tensor_tensor(
                            out=oh[:, :ks, :],
                            in0=dlv.unsqueeze(-1).to_broadcast([P, ks, P]),
                            in1=iota[:].unsqueeze(1).to_broadcast([P, ks, P]),
                            op=mybir.AluOpType.is_equal)
                    ul = pb.tile([P, NBMAX, 2], f16, tag="ul")
                    nc.vector.scalar_tensor_tensor(
                        out=ul[:, :nb, :], in0=u[:, :nb, :], scalar=NEG_SLOPE,
                        in1=u[:, :nb, :],
                        op0=mybir.AluOpType.mult, op1=mybir.AluOpType.max)
                    wexp = pb.tile([P, NBMAX, 2], f16, tag="wexp")
                    nc.scalar.activation(
                        out=wexp[:, :nb, :], in_=ul[:, :nb, :],
                        func=mybir.ActivationFunctionType.Exp)
                    msg = pb.tile([P, NBMAX, 2, 65], f16, tag="msg")
                    nc.vector.tensor_tensor(
                        out=msg[:, :nb, :, :],
                        in0=wexp[:, :nb, :].unsqueeze(-1).to_broadcast(
                            [P, nb, 2, 65]),
                        in1=gt[:, go:go + nb, :, 0:65],
                        op=mybir.AluOpType.mult)
                    ps = pbp.tile([P, 65], f32, tag=f"ps{cv}")
                    psums.append(ps)
                    # self-loop message (host-computed) seeds the psum
                    nc.tensor.matmul(
                        out=ps[:], lhsT=eye[:], rhs=slft[:, s, cv, 0:65],
                        start=True, stop=(nb == 0))
                    for j in range(nb):
                        lhsT = eye[:] if j < qs else oh[:, j - qs, :]
                        for h in range(2):
                            nc.tensor.matmul(
                                out=ps[:], lhsT=lhsT, rhs=msg[:, j, h, :],
                                start=False,
                                stop=(j == nb - 1 and h == 1))
                # finalize
                os_ = []
                for cv in range(2):
                    ps = psums[cv]
                    den = pb.tile([P, 1], f32, tag="den")
                    nc.vector.tensor_scalar_max(den[:], ps[:, 64:65], 1e-30)
                    rec = pb.tile([P, 1], f32, tag="rec")
                    nc.vector.reciprocal(out=rec[:], in_=den[:])
                    rec2 = pb.tile([P, 1], f32, tag="rec2")
                    nc.vector.tensor_scalar_mul(
                        rec2[:], rec[:], (1.0 - ALPHA) if cv == 0 else ALPHA)
                    o = pb.tile([P, DOUT], f32, tag=f"o{cv}")
                    nc.scalar.mul(out=o[:], in_=ps[:, 0:64], mul=rec2[:])
                    os_.append(o)
                ofin = pb.tile([P, DOUT], f32, tag="ofin")
                nc.vector.tensor_tensor(
                    out=ofin[:], in0=os_[0][:], in1=os_[1][:],
                    op=mybir.AluOpType.add)
                nc.vector.tensor_tensor(
                    out=ofin[:], in0=ofin[:], in1=bcomb[:],
                    op=mybir.AluOpType.add)
                nc.sync.dma_start(out=out_d[s * P:(s + 1) * P, :],
                                  in_=ofin[:])


def _build(meta, stub=False, reps=1):
    nc = bacc.Bacc("TRN2", target_bir_lowering=False, debug=False,
                   num_devices=NCORES, dynamic_dma_scratch_size=SCRATCH,
                   num_swdge_queues=NQ)
    q1, kbt1, q2, kbt2 = meta
    nb1 = sum(q1) + sum(kbt1)
    nb2 = sum(q2) + sum(kbt2)
    ins = {
        "xT": nc.dram_tensor("xT", [DIN, NT], f16, kind="ExternalInput").ap(),
        "wfull": nc.dram_tensor("wfull", [DIN, WCOLS], f16,
                                kind="ExternalInput").ap(),
        "iota": nc.dram_tensor("iota", [P, P], f16, kind="ExternalInput").ap(),
        "eye": nc.dram_tensor("eye", [P, P], f16, kind="ExternalInput").ap(),
        "bcomb": nc.dram_tensor("bcomb", [P, DOUT], f32,
                                kind="ExternalInput").ap(),
        "slf": nc.dram_tensor("slf", [P, CPC * 2 * 66], f16,
                              kind="ExternalInput").ap(),
    }
    for cv, (nb, kbt) in enumerate(((nb1, kbt1), (nb2, kbt2))):
        kt = max(sum(kbt), 1)
        ins[f"ix{cv + 1}"] = nc.dram_tensor(
            f"ix{cv + 1}", [P, nb * 8], i16, kind="ExternalInput").ap()
        ins[f"par{cv + 1}"] = nc.dram_tensor(
            f"par{cv + 1}", [P, nb * 2], f16, kind="ExternalInput").ap()
        ins[f"dlt{cv + 1}"] = nc.dram_tensor(
            f"dlt{cv + 1}", [P, kt], f16, kind="ExternalInput").ap()
    outs = {"out": nc.dram_tensor("out", [NPC, DOUT], f32,
                                  kind="ExternalOutput").ap()}
    with tile.TileContext(nc) as tc:
        if stub:
            with tc.tile_pool(name="s", bufs=1) as p:
                t = p.tile([P, DOUT], f32)
                tc.nc.sync.dma_start(out=t[:], in_=ins["bcomb"][:])
                tc.nc.sync.dma_start(out=outs["out"][0:P, :], in_=t[:])
        else:
            _emit(tc, outs, ins, meta, reps=reps)
    nc.compile()
    return nc


# ------------------------------------------------------------------- entry
def kernel(x, edge_index, W1, att_src1, att_dst1, b1,
           W2, att_src2, att_dst2, b2):
    common, per_core, meta = _preprocess(
        np.asarray(x), np.asarray(edge_index),
        np.asarray(W1, np.float64), np.asarray(att_src1, np.float64),
        np.asarray(att_dst1, np.float64), np.asarray(b1, np.float32),
        np.asarray(W2, np.float64), np.asarray(att_src2, np.float64),
        np.asarray(att_dst2, np.float64), np.asarray(b2, np.float32))

    if meta not in _CACHE:
        _CACHE[meta] = _build(meta)
    nc = _CACHE[meta]

    in_maps = [dict(common, **pc) for pc in per_core]
    res = bass_utils.run_bass_kernel_spmd(
        nc, in_maps, core_ids=list(range(NCORES)))
    full = np.concatenate(
        [res.results[k]["out"] for k in range(NCORES)], axis=0)
    return np.ascontiguousarray(full[:N]).astype(np.float32)



# revision 49
# speedup vs baseline: 1.0541x; 1.0541x over previous
"""DirGATConv on 8 Trainium2 NeuronCores (Bass/Tile), v3.

Problem: nn_DirGATConv  (N=50000 nodes, E=800000 edges, DIN=128, DOUT=64)
    out = 0.5 * GATConv(x, src->dst, W1) + 0.5 * GATConv(x, dst->src, W2)

The kernel is gather-descriptor bound (SWDGE runs on 4 Pool queues with
~2 DMA engines each; per-descriptor cost barely depends on 256B vs 512B
element size), so the design minimizes gather descriptors:
  * Self-edges (incl. the PyG-added self-loop) never hit the gather path:
    their messages are fully host-computed (slf input) and seeded into the
    psum with one eye-matmul per (chunk, conv).
  * Each core gets its own table-row permutation (per-core xT column
    order).  A per-core greedy matching pairs sources that share a
    destination in either conv onto adjacent rows (2k, 2k+1), so one 512B
    pair-descriptor serves TWO edges (both halves active); ~12% of cells
    merge this way.
  * a_d[dst] is computed on host (f64) and folded into the par table, so
    the device never materializes attention-dst terms; par is a_d for an
    active half and -200 (kills exp) for inactive/padding halves.
  * Phase A (replicated): one fused matmul per chunk computes
    [xw1 | . | a_s1 | xw2 | . | a_s2]; ACT/DVE copies convert PSUM->f16
    staging; tables stored row-permuted (row = (col%128)*G + col//128) so
    staged writes are >=512B/descriptor.  Row = [xw(64) | 1 | a_s | junk].
  * Phase B: per (chunk, conv) an int16 index space addresses row PAIRS
    (idx = row>>1, elem 512B).  Each dst's first q cells sit in "diagonal"
    slots (partition = dst local id, lhsT = I); leftovers go to one-hot
    "tail" blocks.  u = a_s(gathered) + par; exp(leaky_relu(u)) runs on
    ACT over the per-cell scalars only; messages [w*xw | w] accumulate on
    the PE into PSUM [128, 65] per conv; column 64 is the softmax
    denominator.
"""

import math

import numpy as np

import concourse.bass as bass
import concourse.mybir as mybir
import concourse.tile as tile
from concourse import bacc, bass_utils
from concourse._compat import with_exitstack

# ---------------------------------------------------------------- constants
N = 50000
E = 800000
DIN = 128
DOUT = 64
ALPHA = 0.5
NEG_SLOPE = 0.2
NCORES = 8
P = 128

G = NCORES * math.ceil(math.ceil(N / P) / NCORES)  # 392 padded chunks
CPC = G // NCORES                                  # 49 chunk slots per core
NT = G * P                                         # 50176 padded node count
NPC = CPC * P                                      # 6272 nodes per core

import os

WCOLS = 132        # wfull columns: 2 convs x [W(64) | 1 | a_s]
RNDA = 3           # chunks per phase-A psum round (3*132*4B < 2KB bank)
STG = 12           # chunks per staging flush
SCN = int(os.environ.get("K_SCN", 2))    # chunks per gather super-tile
QCAP = int(os.environ.get("K_QCAP", 8))  # blocks per gather call
SCRATCH = int(os.environ.get("K_SCRATCH", 16384))
                   # SWDGE descriptor carveout bytes (1024 descs);
                   # larger values fail to execute under this runtime
NQ = int(os.environ.get("K_NQ", 4))      # SWDGE queues, round-robin
SP = os.environ.get("K_SP", "1") == "1"  # dma_gather single_packet
PGB = int(os.environ.get("K_PGB", 2))    # gather tile pool bufs

f32 = mybir.dt.float32
f16 = mybir.dt.float16
i16 = mybir.dt.int16

VARIANT = "full"    # "full" | "gathers" | "phasea"  (perf-bisect variants)

_CACHE = {}


# ------------------------------------------------------------ host preprocess
def _match_cores(dsts, srcs):
    """Per-core greedy pairing of sources that share a destination (over
    both convs).  Matched pairs get adjacent table rows (2k, 2k+1) so one
    512B pair-descriptor serves two edges.  Returns (rowOf [NCORES, NT],
    nodeAt [NCORES, NT]): table row of each node / node at each xT column,
    per core."""
    key_all = np.concatenate([dsts, srcs + NT])      # disjoint group spaces
    gidx_all = np.concatenate([srcs, dsts])
    core_all = (np.concatenate([dsts, srcs]) // P) // CPC
    rowOf = np.empty((NCORES, NT), np.int64)
    nodeAt = np.empty((NCORES, NT), np.int64)
    jcols = np.arange(G * P)
    rows_of_col = (jcols % P) * G + jcols // P
    for c in range(NCORES):
        m = core_all == c
        g, s = key_all[m], gidx_all[m]
        order = np.lexsort((s, g))
        gs, ss = g[order], s[order]
        starts = np.flatnonzero(np.r_[True, gs[1:] != gs[:-1]])
        ends = np.r_[starts[1:], len(gs)]
        pa, pb = [], []
        for st, en in zip(starts, ends):
            if en - st < 2:
                continue
            mem = np.unique(ss[st:en])
            if len(mem) < 2:
                continue
            ii, jj = np.triu_indices(len(mem), 1)
            pa.append(mem[ii])
            pb.append(mem[jj])
        A = np.empty(0, np.int64)
        B = np.empty(0, np.int64)
        if pa:
            pa = np.concatenate(pa)
            pb = np.concatenate(pb)
            pk = np.minimum(pa, pb) * np.int64(NT) + np.maximum(pa, pb)
            up, cnt = np.unique(pk, return_counts=True)
            o = np.argsort(-cnt, kind="stable")
            la, lb = up[o] // NT, up[o] % NT
            used = np.zeros(NT, bool)
            keep = np.zeros(len(la), bool)
            for i in range(len(la)):
                a, b = la[i], lb[i]
                if not (used[a] or used[b]):
                    used[a] = used[b] = True
                    keep[i] = True
            A, B = la[keep], lb[keep]
        npair = len(A)
        i = np.arange(npair)
        p, gp = i % P, i // P
        na = np.empty(G * P, np.int64)
        taken = np.zeros(G * P, bool)
        ca = (2 * gp) * P + p
        cb = (2 * gp + 1) * P + p
        na[ca] = A
        na[cb] = B
        taken[ca] = True
        taken[cb] = True
        isn = np.zeros(NT, bool)
        isn[A] = True
        isn[B] = True
        na[np.flatnonzero(~taken)] = np.flatnonzero(~isn)
        nodeAt[c] = na
        rowOf[c, na] = rows_of_col
    return rowOf, nodeAt


def _prep_conv(key, gidx, adv, rowOf):
    """Edge layout for one conv.  key = group node (output row), gidx =
    gathered node, adv = host-computed a_d per node (folded into par),
    rowOf = per-core table row of each node.  Edges sharing (dst, row-pair)
    merge into one cell (both 256B halves active).
    Returns (q, kbt, per-core dict arrays)."""
    key = np.asarray(key, np.int64)
    gidx = np.asarray(gidx, np.int64)
    ecore = (key // P) // CPC
    row = rowOf[ecore, gidx]
    kpr = row >> 1
    parity = (row & 1).astype(np.int64)

    # rank within (key, kpr, parity) splits duplicate edges into cells
    okey = (key * 32768 + kpr) * 2 + parity
    order = np.argsort(okey, kind="stable")
    so = okey[order]
    newrun = np.r_[True, so[1:] != so[:-1]]
    runid = np.cumsum(newrun) - 1
    rstarts = np.flatnonzero(newrun)
    rank = np.empty(len(okey), np.int64)
    rank[order] = np.arange(len(so)) - rstarts[runid]
    RMAX = int(rank.max()) + 1

    ckey = (key * 32768 + kpr) * RMAX + rank
    ucells, inv = np.unique(ckey, return_inverse=True)
    cdst = ucells // (32768 * RMAX)
    cpair = (ucells // RMAX) % 32768
    chunk = cdst // P
    dloc = cdst % P
    core = chunk // CPC
    slot = chunk % CPC

    D = np.bincount(chunk * P + dloc, minlength=G * P).reshape(
        NCORES, CPC, P)

    TAILW = 0.3
    q = np.zeros(CPC, np.int64)
    kbt = np.zeros(CPC, np.int64)
    for s in range(CPC):
        Ds = D[:, s, :]
        best = None
        for qq in range(0, int(Ds.max()) + 1):
            tails = np.maximum(Ds - qq, 0).sum(axis=1)
            kb = int(np.ceil(tails / P).max())
            cost = qq + (1.0 + TAILW) * kb
            if best is None or cost < best[0] or (
                    cost == best[0] and qq > best[1]):
                best = (cost, qq, kb)
        q[s], kbt[s] = best[1], best[2]

    nb = q + kbt                       # blocks per (slot, conv)
    boff = np.zeros(CPC + 1, np.int64)
    boff[1:] = np.cumsum(nb)
    tboff = np.zeros(CPC + 1, np.int64)
    tboff[1:] = np.cumsum(kbt)
    NB = int(boff[-1])
    KT = int(tboff[-1])

    # rank of each cell within its (chunk, dloc) group
    ck = chunk * P + dloc
    order = np.argsort(ck, kind="stable")
    counts = np.bincount(ck, minlength=G * P)
    gstart = np.zeros(G * P, np.int64)
    gstart[1:] = np.cumsum(counts)[:-1]
    crank = np.empty(len(ck), np.int64)
    crank[order] = np.arange(len(ck)) - gstart[ck[order]]

    qs_e = q[slot]
    isdiag = crank < qs_e

    tkey = core * CPC + slot
    torder = np.argsort(np.where(isdiag, -1, tkey), kind="stable")
    tsorted = torder[int(isdiag.sum()):]          # tail cells, grouped
    tcounts = np.bincount(tkey[tsorted], minlength=NCORES * CPC)
    tstart = np.zeros(NCORES * CPC, np.int64)
    tstart[1:] = np.cumsum(tcounts)[:-1]
    trank = np.zeros(len(ck), np.int64)
    trank[tsorted] = np.arange(len(tsorted)) - tstart[tkey[tsorted]]

    blk = np.where(isdiag, crank, qs_e + trank // P)
    lane = np.where(isdiag, dloc, trank % P)
    cgpos = (boff[slot] + blk) * P + lane

    TOT = NB * P
    ix = np.zeros((NCORES, TOT), np.int16)
    par = np.full((NCORES, TOT, 2), -200.0, np.float16)
    dlt = np.zeros((NCORES, P, max(KT, 1)), np.float16)

    ix[core, cgpos] = cpair.astype(np.int16)
    par[ecore, cgpos[inv], parity] = adv[key]
    tm = ~isdiag
    dlt[core[tm], lane[tm], tboff[slot[tm]] + trank[tm] // P] = \
        dloc[tm].astype(np.float16)

    # wrap indices for dma_gather: [128, TOT//16]
    ixw = ix.reshape(NCORES, TOT // 16, 16).transpose(0, 2, 1)
    ixw = np.ascontiguousarray(np.tile(ixw, (1, 8, 1)))
    # par device layout [128, NB, 2]
    parw = np.ascontiguousarray(
        par.reshape(NCORES, NB, P, 2).transpose(0, 2, 1, 3)
        .reshape(NCORES, P, NB * 2))
    return (tuple(int(v) for v in q), tuple(int(v) for v in kbt),
            ixw, parw, dlt)


def _preprocess(x, edge_index, W1, att_src1, att_dst1, b1,
                W2, att_src2, att_dst2, b2):
    src = np.asarray(edge_index[0], np.int64)
    dst = np.asarray(edge_index[1], np.int64)
    # self-edges (incl. the PyG-added self-loop) are handled host-side via
    # slf; only cross edges go through the gather path.
    cross = src != dst
    nself = np.bincount(src[~cross], minlength=N)  # real (n,n) multiplicity
    src, dst = src[cross], dst[cross]

    x64 = np.asarray(x, np.float64)
    W1_, W2_ = np.asarray(W1, np.float64), np.asarray(W2, np.float64)
    xw1 = x64 @ W1_
    xw2 = x64 @ W2_
    as1 = xw1 @ np.asarray(att_src1, np.float64)
    as2 = xw2 @ np.asarray(att_src2, np.float64)
    ad1 = xw1 @ np.asarray(att_dst1, np.float64)
    ad2 = xw2 @ np.asarray(att_dst2, np.float64)

    rowOf, nodeAt = _match_cores(dst, src)
    q1, kbt1, ixw1, parw1, dlt1 = _prep_conv(dst, src, ad1, rowOf)
    q2, kbt2, ixw2, parw2, dlt2 = _prep_conv(src, dst, ad2, rowOf)

    # per-node self message  (1+m) * exp(lrelu(a_s+a_d)) * [xw | 1]
    def lrelu(v):
        return np.where(v > 0, v, NEG_SLOPE * v)

    slf = np.zeros((NCORES, P, CPC, 2, 66), np.float16)
    for cv, (xw, a_s, a_d) in enumerate(((xw1, as1, ad1), (xw2, as2, ad2))):
        ws = (1.0 + nself) * np.exp(lrelu(a_s + a_d))        # [N]
        m = np.zeros((NT, 66))
        m[:N, :64] = ws[:, None] * xw
        m[:N, 64] = ws
        mg = m.reshape(G, P, 66).transpose(1, 0, 2)          # [P, G, 66]
        slf[:, :, :, cv, :] = mg.reshape(
            P, NCORES, CPC, 66).transpose(1, 0, 2, 3).astype(np.float16)

    xT0 = np.zeros((DIN, NT), np.float16)
    xT0[:, :N] = np.asarray(x, np.float32).T.astype(np.float16)

    wfull = np.zeros((DIN, WCOLS), np.float64)
    for cv, (W, a_s) in enumerate(((W1, att_src1), (W2, att_src2))):
        o = cv * 66
        wfull[:, o:o + 64] = W
        wfull[:, o + 65] = W @ a_s
    wfull = wfull.astype(np.float16)

    iota = np.broadcast_to(np.arange(P, dtype=np.float16), (P, P)).copy()
    eye = np.eye(P, dtype=np.float16)
    bcomb = np.broadcast_to(
        ((1.0 - ALPHA) * np.asarray(b1, np.float64)
         + ALPHA * np.asarray(b2, np.float64)).astype(np.float32),
        (P, DOUT)).copy()

    common = dict(wfull=wfull, iota=iota, eye=eye, bcomb=bcomb)
    per_core = []
    for k in range(NCORES):
        per_core.append(dict(
            xT=np.ascontiguousarray(xT0[:, nodeAt[k]]),
            ix1=ixw1[k], ix2=ixw2[k],
            par1=parw1[k], par2=parw2[k],
            dlt1=dlt1[k], dlt2=dlt2[k],
            slf=np.ascontiguousarray(slf[k].reshape(P, CPC * 2 * 66))))
    meta = (q1, kbt1, q2, kbt2)
    return common, per_core, meta


# ------------------------------------------------------------- device program
@with_exitstack
def _emit(ctx, tc, outs, ins, meta, reps=1):
    nc = tc.nc
    t1_d = nc.dram_tensor("T1_tab", [NT, P], f16, kind="Internal").ap()
    t2_d = nc.dram_tensor("T2_tab", [NT, P], f16, kind="Internal").ap()
    for _ in range(reps):
        _emit_iter(tc, outs, ins, meta, t1_d, t2_d)


def _emit_iter(tc, outs, ins, meta, t1_d, t2_d):
    nc = tc.nc
    out_d = outs["out"]
    q1, kbt1, q2, kbt2 = meta
    qs_ = (q1, q2)
    kbt_ = (kbt1, kbt2)
    nb_ = tuple(tuple(a + b for a, b in zip(qs_[c], kbt_[c]))
                for c in range(2))
    boff_ = []
    tboff_ = []
    for c in range(2):
        bo = [0]
        to = [0]
        for s in range(CPC):
            bo.append(bo[-1] + nb_[c][s])
            to.append(to[-1] + kbt_[c][s])
        boff_.append(bo)
        tboff_.append(to)
    NBMAX = max(max(nb_[0]), max(nb_[1]))
    KTMAX = max(max(kbt_[0]), max(kbt_[1]), 1)

    t_views = [t.rearrange("(p g) c -> p g c", p=P) for t in (t1_d, t2_d)]
    t_pair = [t.rearrange("(r t) c -> r (t c)", t=2) for t in (t1_d, t2_d)]

    # ---------------- phase A: tables ----------------
    with tc.tile_pool(name="pa0", bufs=1) as pa0, \
         tc.tile_pool(name="pa", bufs=2) as pa, \
         tc.tile_pool(name="pastg", bufs=2) as pastg, \
         tc.tile_pool(name="pap", bufs=2, space="PSUM") as pap:
        wf = pa0.tile([P, WCOLS], f16)
        nc.sync.dma_start(out=wf[:], in_=ins["wfull"][:])

        for piece in range(NCORES):
            xt = pa.tile([P, NPC], f16, tag="xt")
            # split the load across HWDGE and SWDGE paths (balances phase-A
            # DMA: each path ends up with ~19MB incl. the staging writes)
            h = NPC // 2
            nc.sync.dma_start(
                out=xt[:, :h],
                in_=ins["xT"][:, piece * NPC:piece * NPC + h])
            nc.gpsimd.dma_start(
                out=xt[:, h:],
                in_=ins["xT"][:, piece * NPC + h:(piece + 1) * NPC])
            stg = [pastg.tile([P, CPC, P], f16, tag=f"stg{t}",
                              name=f"stg{t}") for t in range(2)]
            gbase = piece * CPC
            for j0 in range(0, CPC, RNDA):
                r = min(RNDA, CPC - j0)
                ps = pap.tile([P, RNDA * WCOLS], f32, tag="pap")
                for k in range(r):
                    nc.tensor.matmul(
                        out=ps[:, k * WCOLS:(k + 1) * WCOLS],
                        lhsT=xt[:, (j0 + k) * P:(j0 + k + 1) * P],
                        rhs=wf[:], start=True, stop=True)
                psv = ps[:].rearrange("p (k c) -> p k c", k=RNDA)
                # T1 copy on ACT, T2 on DVE - balance the engines
                nc.scalar.copy(
                    out=stg[0][:, j0:j0 + r, 0:66], in_=psv[:, :r, 0:66])
                nc.vector.tensor_copy(
                    out=stg[1][:, j0:j0 + r, 0:66], in_=psv[:, :r, 66:132])
                for t in range(2):
                    nc.vector.memset(stg[t][:, j0:j0 + r, 64:65], 1.0)
            # T1 via HWDGE (SP/Act), T2 via SWDGE (Pool) - the SWDGE DMA
            # engines are idle during phase A, so this doubles write paths
            nc.sync.dma_start(
                out=t_views[0][:, gbase:gbase + CPC, :],
                in_=stg[0][:, :, :])
            nc.gpsimd.dma_start(
                out=t_views[1][:, gbase:gbase + CPC, :],
                in_=stg[1][:, :, :])

    # ---------------- phase B: edge aggregation ----------------
    with tc.tile_pool(name="pb0", bufs=1) as pb0, \
         tc.tile_pool(name="pg", bufs=PGB) as pg, \
         tc.tile_pool(name="pb", bufs=3) as pb, \
         tc.tile_pool(name="pbp", bufs=int(os.environ.get("K_PBP", 4)),
                      space="PSUM") as pbp:
        iota = pb0.tile([P, P], f16)
        nc.sync.dma_start(out=iota[:], in_=ins["iota"][:])
        eye = pb0.tile([P, P], f16)
        nc.sync.dma_start(out=eye[:], in_=ins["eye"][:])
        bcomb = pb0.tile([P, DOUT], f32)
        nc.sync.dma_start(out=bcomb[:], in_=ins["bcomb"][:])
        slft = pb0.tile([P, CPC, 2, 66], f16)
        nc.sync.dma_start(out=slft[:].rearrange("p s c v -> p (s c v)"),
                          in_=ins["slf"][:])
        dlts = []
        pars = []
        ixalls = []
        for cv in range(2):
            kt = max(tboff_[cv][-1], 1)
            t = pb0.tile([P, kt], f16, tag=f"dlt{cv}", name=f"dlt{cv}")
            nc.sync.dma_start(out=t[:], in_=ins[f"dlt{cv + 1}"][:])
            dlts.append(t)
            nbt = boff_[cv][-1]
            t = pb0.tile([P, nbt, 2], f16, tag=f"par{cv}", name=f"par{cv}")
            nc.sync.dma_start(
                out=t[:].rearrange("p b t -> p (b t)"),
                in_=ins[f"par{cv + 1}"][:])
            pars.append(t)
            # all gather indices resident up front: kills the per-super-tile
            # ixt-load dependency ahead of each gather batch
            t = pb0.tile([P, nbt * 8], i16, tag=f"ixall{cv}",
                         name=f"ixall{cv}")
            nc.sync.dma_start(out=t[:], in_=ins[f"ix{cv + 1}"][:])
            ixalls.append(t)

        if VARIANT == "phasea":
            for s in range(CPC):
                nc.sync.dma_start(out=out_d[s * P:(s + 1) * P, :],
                                  in_=bcomb[:])
            return

        rrq = [0]
        NBS = [max(sum(nb_[cv][s0:s0 + SCN]) for s0 in range(0, CPC, SCN))
               for cv in range(2)]
        t_quad = [t.rearrange("(r t) c -> r (t c)", t=4) for t in (t1_d, t2_d)]

        for s0 in range(0, CPC, SCN):
            scr = min(SCN, CPC - s0)
            gts = []
            for cv in range(2):
                nbsup = sum(nb_[cv][s0:s0 + scr])
                bo0 = boff_[cv][s0]
                gt = None
                if VARIANT not in ("g256", "g1024"):
                    gt = pg.tile([P, NBS[cv], 2, P], f16, tag=f"gt{cv}",
                                 name=f"gt{cv}")
                gv = (gt[:].rearrange("p k t c -> p k (t c)")
                      if gt is not None else None)
                if VARIANT == "g256":
                    # timing probe: same desc count, 256B elems
                    g2 = pg.tile([P, NBS[cv], P], f16, tag=f"g2{cv}",
                                 name=f"g2{cv}")
                    for b0 in range(0, nbsup, QCAP):
                        bw = min(QCAP, nbsup - b0)
                        nc.gpsimd.dma_gather(
                            out_ap=g2[:, b0:b0 + bw, :],
                            in_ap=(t1_d, t2_d)[cv],
                            idxs_ap=ixalls[cv][:, (bo0 + b0) * 8:
                                               (bo0 + b0 + bw) * 8],
                            num_idxs=bw * P, num_idxs_reg=bw * P,
                            elem_size=P, queue_num=rrq[0])
                        rrq[0] = (rrq[0] + 1) % NQ
                elif VARIANT == "g1024":
                    # timing probe: half desc count, 1024B elems.
                    # Mask idx to < NT//4 so quad-view reads stay in bounds.
                    ixm = pg.tile([P, NBS[cv] * 8], i16, tag=f"ixm{cv}",
                                  name=f"ixm{cv}")
                    nc.vector.tensor_scalar(
                        out=ixm[:, :nbsup * 8],
                        in0=ixalls[cv][:, bo0 * 8:(bo0 + nbsup) * 8],
                        scalar1=0x1FFF, scalar2=None,
                        op0=mybir.AluOpType.bitwise_and)
                    gq = pg.tile([P, NBS[cv] // 2 + 1, 4 * P], f16,
                                 tag=f"gq{cv}", name=f"gq{cv}")
                    for b0 in range(0, nbsup // 2, QCAP):
                        bw = min(QCAP, nbsup // 2 - b0)
                        nc.gpsimd.dma_gather(
                            out_ap=gq[:, b0:b0 + bw, :], in_ap=t_quad[cv],
                            idxs_ap=ixm[:, b0 * 8:(b0 + bw) * 8],
                            num_idxs=bw * P, num_idxs_reg=bw * P,
                            elem_size=4 * P, queue_num=rrq[0])
                        rrq[0] = (rrq[0] + 1) % NQ
                else:
                    for b0 in range(0, nbsup, QCAP):
                        bw = min(QCAP, nbsup - b0)
                        nc.gpsimd.dma_gather(
                            out_ap=gv[:, b0:b0 + bw, :], in_ap=t_pair[cv],
                            idxs_ap=ixalls[cv][:, (bo0 + b0) * 8:
                                               (bo0 + b0 + bw) * 8],
                            num_idxs=bw * P, num_idxs_reg=bw * P,
                            elem_size=2 * P, queue_num=rrq[0],
                            single_packet=SP)
                        rrq[0] = (rrq[0] + 1) % NQ
                gts.append(gt if VARIANT != "g256" else None)
            if VARIANT != "full":
                for s in range(s0, s0 + scr):
                    nc.sync.dma_start(out=out_d[s * P:(s + 1) * P, :],
                                      in_=bcomb[:])
                continue
            for s in range(s0, s0 + scr):
                psums = []
                for cv in range(2):
                    qs = qs_[cv][s]
                    ks = kbt_[cv][s]
                    nb = nb_[cv][s]
                    bo = boff_[cv][s]
                    to = tboff_[cv][s]
                    go = boff_[cv][s] - boff_[cv][s0]
                    gt = gts[cv]
                    # par carries host-computed a_d[dst] (or -200 disable)
                    u = pb.tile([P, NBMAX, 2], f16, tag="u")
                    nc.vector.tensor_tensor(
                        out=u[:, :nb, :], in0=gt[:, go:go + nb, :, 65],
                        in1=pars[cv][:, bo:bo + nb, :],
                        op=mybir.AluOpType.add)
                    oh = None
                    if ks:
                        oh = pb.tile([P, KTMAX, P], f16, tag="oh")
                        dlv = dlts[cv][:, to:to + ks]
                        nc.vector.tensor_tensor(
                            out=oh[:, :ks, :],
                            in0=dlv.unsqueeze(-1).to_broadcast([P, ks, P]),
                            in1=iota[:].unsqueeze(1).to_broadcast([P, ks, P]),
                            op=mybir.AluOpType.is_equal)
                    ul = pb.tile([P, NBMAX, 2], f16, tag="ul")
                    nc.vector.scalar_tensor_tensor(
                        out=ul[:, :nb, :], in0=u[:, :nb, :], scalar=NEG_SLOPE,
                        in1=u[:, :nb, :],
                        op0=mybir.AluOpType.mult, op1=mybir.AluOpType.max)
                    wexp = pb.tile([P, NBMAX, 2], f16, tag="wexp")
                    nc.scalar.activation(
                        out=wexp[:, :nb, :], in_=ul[:, :nb, :],
                        func=mybir.ActivationFunctionType.Exp)
                    msg = pb.tile([P, NBMAX, 2, 65], f16, tag="msg")
                    nc.vector.tensor_tensor(
                        out=msg[:, :nb, :, :],
                        in0=wexp[:, :nb, :].unsqueeze(-1).to_broadcast(
                            [P, nb, 2, 65]),
                        in1=gt[:, go:go + nb, :, 0:65],
                        op=mybir.AluOpType.mult)
                    ps = pbp.tile([P, 65], f32, tag=f"ps{cv}")
                    psums.append(ps)
                    # self-loop message (host-computed) seeds the psum
                    nc.tensor.matmul(
                        out=ps[:], lhsT=eye[:], rhs=slft[:, s, cv, 0:65],
                        start=True, stop=(nb == 0))
                    for j in range(nb):
                        lhsT = eye[:] if j < qs else oh[:, j - qs, :]
                        for h in range(2):
                            nc.tensor.matmul(
                                out=ps[:], lhsT=lhsT, rhs=msg[:, j, h, :],
                                start=False,
                                stop=(j == nb - 1 and h == 1))
                # finalize
                os_ = []
                for cv in range(2):
                    ps = psums[cv]
                    den = pb.tile([P, 1], f32, tag="den")
                    nc.vector.tensor_scalar_max(den[:], ps[:, 64:65], 1e-30)
                    rec = pb.tile([P, 1], f32, tag="rec")
                    nc.vector.reciprocal(out=rec[:], in_=den[:])
                    rec2 = pb.tile([P, 1], f32, tag="rec2")
                    nc.vector.tensor_scalar_mul(
                        rec2[:], rec[:], (1.0 - ALPHA) if cv == 0 else ALPHA)
                    o = pb.tile([P, DOUT], f32, tag=f"o{cv}")
                    nc.scalar.mul(out=o[:], in_=ps[:, 0:64], mul=rec2[:])
                    os_.append(o)
                ofin = pb.tile([P, DOUT], f32, tag="ofin")
                nc.vector.tensor_tensor(
                    out=ofin[:], in0=os_[0][:], in1=os_[1][:],
                    op=mybir.AluOpType.add)
                nc.vector.tensor_tensor(
                    out=ofin[:], in0=ofin[:], in1=bcomb[:],
                    op=mybir.AluOpType.add)
                nc.sync.dma_start(out=out_d[s * P:(s + 1) * P, :],
                                  in_=ofin[:])


def _build(meta, stub=False, reps=1):
    nc = bacc.Bacc("TRN2", target_bir_lowering=False, debug=False,
                   num_devices=NCORES, dynamic_dma_scratch_size=SCRATCH,
                   num_swdge_queues=NQ)
    q1, kbt1, q2, kbt2 = meta
    nb1 = sum(q1) + sum(kbt1)
    nb2 = sum(q2) + sum(kbt2)
    ins = {
        "xT": nc.dram_tensor("xT", [DIN, NT], f16, kind="ExternalInput").ap(),
        "wfull": nc.dram_tensor("wfull", [DIN, WCOLS], f16,
                                kind="ExternalInput").ap(),
        "iota": nc.dram_tensor("iota", [P, P], f16, kind="ExternalInput").ap(),
        "eye": nc.dram_tensor("eye", [P, P], f16, kind="ExternalInput").ap(),
        "bcomb": nc.dram_tensor("bcomb", [P, DOUT], f32,
                                kind="ExternalInput").ap(),
        "slf": nc.dram_tensor("slf", [P, CPC * 2 * 66], f16,
                              kind="ExternalInput").ap(),
    }
    for cv, (nb, kbt) in enumerate(((nb1, kbt1), (nb2, kbt2))):
        kt = max(sum(kbt), 1)
        ins[f"ix{cv + 1}"] = nc.dram_tensor(
            f"ix{cv + 1}", [P, nb * 8], i16, kind="ExternalInput").ap()
        ins[f"par{cv + 1}"] = nc.dram_tensor(
            f"par{cv + 1}", [P, nb * 2], f16, kind="ExternalInput").ap()
        ins[f"dlt{cv + 1}"] = nc.dram_tensor(
            f"dlt{cv + 1}", [P, kt], f16, kind="ExternalInput").ap()
    outs = {"out": nc.dram_tensor("out", [NPC, DOUT], f32,
                                  kind="ExternalOutput").ap()}
    with tile.TileContext(nc) as tc:
        if stub:
            with tc.tile_pool(name="s", bufs=1) as p:
                t = p.tile([P, DOUT], f32)
                tc.nc.sync.dma_start(out=t[:], in_=ins["bcomb"][:])
                tc.nc.sync.dma_start(out=outs["out"][0:P, :], in_=t[:])
        else:
            _emit(tc, outs, ins, meta, reps=reps)
    nc.compile()
    return nc


# ------------------------------------------------------------------- entry
def kernel(x, edge_index, W1, att_src1, att_dst1, b1,
           W2, att_src2, att_dst2, b2):
    common, per_core, meta = _preprocess(
        np.asarray(x), np.asarray(edge_index),
        np.asarray(W1, np.float64), np.asarray(att_src1, np.float64),
        np.asarray(att_dst1, np.float64), np.asarray(b1, np.float32),
        np.asarray(W2, np.float64), np.asarray(att_src2, np.float64),
        np.asarray(att_dst2, np.float64), np.asarray(b2, np.float32))

    if meta not in _CACHE:
        _CACHE[meta] = _build(meta)
    nc = _CACHE[meta]

    in_maps = [dict(common, **pc) for pc in per_core]
    res = bass_utils.run_bass_kernel_spmd(
        nc, in_maps, core_ids=list(range(NCORES)))
    full = np.concatenate(
        [res.results[k]["out"] for k in range(NCORES)], axis=0)
    return np.ascontiguousarray(full[:N]).astype(np.float32)

